# revision 14
# baseline (speedup 1.0000x reference)
"""MQA attention with ALiBi + causal mask on 8 TRN2 NeuronCores.

Problem: hidden_states [2,2048,4096] @ Wq -> 32 query heads of 128; single
KV head via Wkv; scores + ALiBi bias + causal mask; softmax; @ Wo.

Distribution: q projection and attention are HEAD-sharded (core c owns
heads {c, c+8, c+16, c+24}, one per octile "slot"), so there is NO q
AllToAll: each core computes q for its 4 heads over ALL 4096 tokens
(same FLOPs as token-sharded q). KV projection and the output projection
are TOKEN-sharded (core c owns 256 tokens of each batch). Bridges: tiny
kT/v AllGathers right after the kv projection, and one attnT AllToAll
per (batch, head-half) fired the moment those 16 heads finish, so every
collective hides under compute and the output projection never waits.

Attention: scoresT[kv, q] layout; per-slot ALiBi reach cutoff at
threshold 14 (dropped tail mass ~8e-7): SLOT_D = [56, 224, 896, inf].
Softmax shift-invariance: the per-q-column ALiBi term exp(-slope*f)
cancels between numerator and denominator, so off-diagonal chunks of
slots 1-3 need only a PER-PARTITION bias slope*(128*delta+p), folded
into the exp activation's bias operand -- no DVE bias op at all. Slot 0
(slopes up to 0.84, column shift up to e^214 would overflow f32) keeps
the exact rel path; diagonal chunks use a masked, -f-free rel' table so
they stay consistent with the bias-path chunks of the same head.
exp without max-subtraction (scores O(10), shifted args bounded by
~e^80 < f32 max); denominator via ALL-ONES [128,128] matmuls whose
output rows are all equal to den -- the weight load uses the normal
full-array FWL path (a [128,1] ones column forced a col_grp=q0 partial
load that serialized against in-flight matmuls, ~100ns/matmul), and the
partition-broadcast of 1/den comes free: reciprocal + multiply on DVE,
no broadcast matmul, no scalar-engine staging copy. The PV/den matmuls
trail the QK matmuls by LAG=7 pipeline entries (3-deep PSUM
accumulators): the PE queue is FIFO, so a PV waiting on its exp would
stall every matmul (and its weight load) queued behind it.

All matmuls bf16 (rel-err ~6e-3 vs budget 2e-2); output staged bf16 and
cast to f32 on the host. Weights/activations are host-pre-tiled so every
DMA is contiguous; the q-projection's block-0 DMAs are issued first and
its matmuls emitted before the kv projection, so the PE starts ~1us in
and the kv DMAs hide behind block-0 compute. PSUM: s2/qproj/outproj
share a 3-buffer [128,512] tag; at/den per-unit accumulators are
[128,256] each with 3 buffers. Wo tile 0 prefetches during attention;
outproj runs an explicit i+1 prefetch chain.
"""
import math
import os

import numpy as np
import ml_dtypes

import concourse.bass as bass
from concourse import bacc
import concourse.mybir as mybir
from concourse.tile import TileContext
from concourse.bass_utils import run_bass_kernel_spmd

B, S, H, NH, HD = 2, 2048, 4096, 32, 128
NC = 8              # cores
TPC = 512           # tokens per core (256 per batch) for kv/out projections
KC = H // 128       # 32 contraction chunks
GQ = 8              # 256-token q blocks per batch
NB = 8              # 512-token blocks across both batches (q projection)
SCALE = HD ** -0.5
# per-slot ALiBi reach: 14/min_slope(octile); tail mass <= exp(-14) ~ 8e-7
SLOT_D = [56.0, 224.0, 896.0, float("inf")]
bf16 = mybir.dt.bfloat16
f32 = mybir.dt.float32
Exp = mybir.ActivationFunctionType.Exp
Copy = mybir.ActivationFunctionType.Copy
MULT = mybir.AluOpType.mult
ADD = mybir.AluOpType.add

_CACHE = {}
LAST_EXEC_NS = None


def _alibi_slopes(n_heads):
    closest_pow2 = 2 ** math.floor(math.log2(n_heads))
    base = 2.0 ** (-(2.0 ** -(math.log2(closest_pow2) - 3)))
    slopes = [base ** i for i in range(1, closest_pow2 + 1)]
    if closest_pow2 != n_heads:
        extra_base = 2.0 ** (-(2.0 ** -(math.log2(2 * closest_pow2) - 3)))
        n_extra = min(closest_pow2, n_heads - closest_pow2)
        slopes += [extra_base ** i for i in range(1, 2 * n_extra + 1, 2)]
    return np.asarray(slopes, dtype=np.float32)


def _j0(g, slot):
    d = SLOT_D[slot]
    if math.isinf(d):
        return 0
    return max(0, math.ceil((256 * g - 127 - d) / 128))


def _build_rel():
    """[128,1024]: col 0 exact delta=-1 (256), 256 exact delta=0 (256),
    512 exact delta=1 RIGHT half f in [128,256) (128) -- the left half of a
    delta=1 chunk is fully causal-masked, so its matmuls are narrowed to
    128 columns; 640 -f-free rel' delta=0 (256), 896 rel' delta=1 right
    (128). Masked entries -30000."""
    rel = np.empty((128, 1024), np.float32)
    p = np.arange(128)[:, None]
    f2 = np.arange(256)[None, :]
    f1 = np.arange(128, 256)[None, :]

    def put(lo, vals, mask):
        r = np.broadcast_to(vals.astype(np.float32), mask.shape).copy()
        r[mask] = -30000.0
        rel[:, lo:lo + mask.shape[1]] = r

    put(0, -128 + p - f2, (-128 + p - f2) > 0)
    put(256, p - f2, (p - f2) > 0)
    put(512, 128 + p - f1, (128 + p - f1) > 0)
    put(640, np.broadcast_to(p, (128, 256)), (p - f2) > 0)
    put(896, np.broadcast_to(128 + p, (128, 128)), (128 + p - f1) > 0)
    return rel.astype(ml_dtypes.bfloat16)


def _build_bias(slopes4):
    """[128, 42] f32: col 14*(s-1)+(delta+14) = slope_s*(128*delta+p) for
    slots 1..3, delta in [-14, -1] (off-diagonal bias-path chunks)."""
    b = np.zeros((128, 42), np.float32)
    p = np.arange(128)
    for s in (1, 2, 3):
        for d in range(-14, 0):
            b[:, 14 * (s - 1) + (d + 14)] = slopes4[s] * (128 * d + p)
    return b


def _build_nc():
    nc = bacc.Bacc(num_devices=NC)
    # host-pre-tiled layouts: every weight/act DMA is contiguous in DRAM
    hsT_my = nc.declare_dram_parameter("hsT_my", [128, KC * TPC], bf16,
                                       isOutput=False)
    hsT_all = nc.declare_dram_parameter("hsT_all", [NB, 128, KC * 512], bf16,
                                        isOutput=False)
    Wq_m = nc.declare_dram_parameter("Wq_m", [4, 128, KC * 128], bf16,
                                     isOutput=False)
    Wkv = nc.declare_dram_parameter("Wkv", [128, KC * 256], bf16,
                                    isOutput=False)
    Wo_t = nc.declare_dram_parameter("Wo_t", [8, 128, KC * 512], bf16,
                                     isOutput=False)
    rel = nc.declare_dram_parameter("rel", [128, 1024], bf16,
                                    isOutput=False)
    slopes = nc.declare_dram_parameter("slopes", [128, 4], f32,
                                       isOutput=False)
    biast = nc.declare_dram_parameter("biast", [128, 42], f32,
                                      isOutput=False)
    out = nc.declare_dram_parameter("out", [TPC, H], bf16, isOutput=True)

    grp = [list(range(NC))]
    with TileContext(nc) as tc:
        with (
            tc.tile_pool(name="dram", bufs=1, space="DRAM") as dram,
            tc.tile_pool(name="const", bufs=1) as const,
            tc.tile_pool(name="psum", bufs=1, space="PSUM") as psum,
            tc.tile_pool(name="lng", bufs=1) as lng,
        ):
            kT_in = dram.tile([128, TPC], bf16)
            kT_ag = dram.tile([128 * NC, TPC], bf16, addr_space="Shared")
            v_in = dram.tile([TPC, 128], bf16)
            v_ag = dram.tile([TPC * NC, 128], bf16, addr_space="Shared")
            # attnT bounce buffers per (batch, head-half)
            a_in = {(b, h): dram.tile([2048, 256], bf16, name=f"a_in{b}{h}")
                    for b in range(2) for h in range(2)}
            a_a2a = {(b, h): dram.tile([2048, 256], bf16, name=f"a_a2a{b}{h}")
                     for b in range(2) for h in range(2)}

            ones_mat = const.tile([128, 128], bf16)
            nc.vector.memset(ones_mat[:], 1.0)

            # ---------------- Phase 1+2: projections ----------------------
            with tc.tile_pool(name="ph1", bufs=1) as ph1:
                # Startup DMAs ride both HW DGE queues: activations on the
                # Sync queue, weights on the Scalar queue (idle until the
                # attention exps), ordered by PE consumption. Wq is s-major
                # so the first matmul group needs only its own 1MB slice.
                Wq_sb = ph1.tile([128, 4, KC, 128], bf16)
                hs0 = ph1.tile([128, KC, 512], bf16, tag="hsblk", bufs=2,
                               name="hs_blk")
                hs_r0 = hsT_all[0].rearrange("p (k t) -> p k t", k=KC)
                for lo, hi in ((0, 2), (2, 4), (4, 8), (8, 16), (16, 32)):
                    nc.sync.dma_start(out=hs0[:, lo:hi, :],
                                      in_=hs_r0[:, lo:hi, :])
                for s in range(4):
                    nc.scalar.dma_start(
                        out=Wq_sb[:, s],
                        in_=Wq_m[s].rearrange("p (k m) -> p k m", k=KC))
                # kv-projection DMAs queue behind and hide under block 0
                hsT_sb = ph1.tile([128, KC, TPC], bf16)
                Wkv_sb = ph1.tile([128, KC, 256], bf16)
                hsT_r = hsT_my.rearrange("p (k t) -> p k t", k=KC)
                Wkv_r = Wkv.rearrange("p (k c) -> p k c", k=KC)
                nc.scalar.dma_start(out=Wkv_sb[:], in_=Wkv_r[:])
                for lo, hi in ((0, 8), (8, 20), (20, 32)):
                    sl = slice(lo, hi)
                    nc.sync.dma_start(out=hsT_sb[:, sl, :], in_=hsT_r[:, sl, :])
                rel_sb = const.tile([128, 1024], bf16)
                slopes_sb = const.tile([128, 4], f32)
                bias_sb = const.tile([128, 42], f32)

                # qT[b, s]: [128 d, 8 g-blocks, 256 tok]
                qT = {(b, s): lng.tile([128, GQ, 256], bf16,
                                       name=f"qT_{b}_{s}")
                      for b in range(2) for s in range(4)}
                kT_b, v_b = {}, {}

                def load_kv_sbuf():
                    nc.scalar.dma_start(out=rel_sb[:], in_=rel[:])
                    nc.scalar.dma_start(out=slopes_sb[:], in_=slopes[:])
                    nc.scalar.dma_start(out=bias_sb[:], in_=biast[:])
                    for bb in range(B):
                        t = lng.tile([128, 8, 256], bf16, name=f"kT_{bb}")
                        nc.scalar.dma_start(
                            out=t[:],
                            in_=kT_ag.rearrange("(r p) (b t) -> b p r t",
                                                p=128, b=2)[bb])
                        kT_b[bb] = t
                        t = lng.tile([128, 8, 2, 128], bf16, name=f"v_{bb}")
                        for u in range(2):
                            nc.scalar.dma_start(
                                out=t[:, :, u, :],
                                in_=v_ag.rearrange(
                                    "(r b u p) d -> b p r u d",
                                    b=2, u=2, p=128)[bb][:, :, u, :])
                        v_b[bb] = t

                def qproj_block(tb, hs_blk):
                    b = tb // 4
                    g2 = tb % 4
                    for s in range(4):
                        q_ps = psum.tile([128, 512], f32, tag="s2", bufs=3,
                                         name="q_ps")
                        for k in range(KC):
                            nc.tensor.matmul(
                                q_ps[:],
                                lhsT=Wq_sb[:, s, k, :],
                                rhs=hs_blk[:, k, :],
                                start=(k == 0), stop=(k == KC - 1))
                        nc.vector.tensor_copy(
                            out=qT[b, s][:, 2 * g2:2 * g2 + 2, :],
                            in_=q_ps[:])

                # block 0 first: PE busy while the kv DMAs stream in
                qproj_block(0, hs0)

                # kv projection (my 512 tokens) + AllGathers
                kT_ps = psum.tile([128, TPC], f32, tag="s2", bufs=3)
                for k in range(KC):
                    nc.tensor.matmul(kT_ps[:], lhsT=Wkv_sb[:, k, 0:128],
                                     rhs=hsT_sb[:, k, :],
                                     start=(k == 0), stop=(k == KC - 1))
                kT_sb = ph1.tile([128, TPC], bf16)
                nc.vector.tensor_copy(out=kT_sb[:], in_=kT_ps[:])
                nc.sync.dma_start(out=kT_in[:], in_=kT_sb[:])
                nc.gpsimd.collective_compute(
                    "AllGather", mybir.AluOpType.bypass, replica_groups=grp,
                    ins=[kT_in[:]], outs=[kT_ag[:]])

                for t4 in range(4):
                    v_ps = psum.tile([128, 128], f32, tag="s2", bufs=3,
                                     name="v_ps")
                    for k in range(KC):
                        nc.tensor.matmul(
                            v_ps[:],
                            lhsT=hsT_sb[:, k, 128 * t4:128 * (t4 + 1)],
                            rhs=Wkv_sb[:, k, 128:256],
                            start=(k == 0), stop=(k == KC - 1))
                    v_sb = ph1.tile([128, 128], bf16, tag="v_sb", bufs=3,
                                    name="v_sb")
                    nc.vector.tensor_copy(out=v_sb[:], in_=v_ps[:])
                    nc.sync.dma_start(out=v_in[128 * t4:128 * (t4 + 1), :],
                                      in_=v_sb[:])
                nc.gpsimd.collective_compute(
                    "AllGather", mybir.AluOpType.bypass, replica_groups=grp,
                    ins=[v_in[:]], outs=[v_ag[:]])

                # remaining q-projection blocks
                for tb in range(1, NB):
                    if tb == 6:
                        load_kv_sbuf()
                    hs_blk = ph1.tile([128, KC, 512], bf16, tag="hsblk",
                                      bufs=2, name="hs_blk")
                    nc.sync.dma_start(
                        out=hs_blk[:],
                        in_=hsT_all[tb].rearrange("p (k t) -> p k t", k=KC))
                    qproj_block(tb, hs_blk)

            # ---------------- Phases 3+4: attention & output projection ----
            with (tc.tile_pool(name="attn", bufs=1) as attn,
                  tc.tile_pool(name="ph4", bufs=1) as ph4):
                aT = {}
                for b in range(B):
                    for s in range(4):
                        aT[b, s] = attn.tile([128, GQ, 256], bf16, tag="aT",
                                             bufs=8, name=f"aT_{b}_{s}")

                def kT_chunk(b, j):
                    return kT_b[b][:, j // 2, 128 * (j % 2):128 * (j % 2 + 1)]

                class AttnUnit:
                    """One (b, s, g) softmax unit, emitted in two stages so
                    the PV/den matmuls trail the QK matmuls by several
                    entries: the PE queue is FIFO, so a PV waiting on its
                    exp would otherwise stall every matmul (and its weight
                    load) queued behind it."""

                    def __init__(self, b, s, g):
                        self.b, self.s, self.g = b, s, g
                        self.nch = 2 * (g + 1)
                        self.j0 = _j0(g, s)
                        js = list(range(self.j0, self.nch))
                        self.pairs = [
                            (js[i], js[i + 1] if i + 1 < len(js) else None)
                            for i in range(0, len(js), 2)]
                        self.expp = {}
                        self.at = psum.tile([128, 256], f32, tag="at",
                                            bufs=3, name="at")
                        self.den = psum.tile([128, 256], f32, tag="den",
                                             bufs=2, name="den")

                    def emit_stage1(self, k):
                        b, s, g = self.b, self.s, self.g
                        ja, jb = self.pairs[k]
                        s2 = psum.tile([128, 512], f32, tag="s2", bufs=3,
                                       name="s2")
                        expp = attn.tile([128, 512], bf16, tag="exp", bufs=11,
                                         name="expp")
                        # per chunk: delta=1 is narrowed to its live right
                        # half (q cols 128:256); kind: bias (exp bias AP) or
                        # stt (rel table col rc)
                        info = []
                        lo = 0
                        for j in (ja, jb):
                            if j is None:
                                continue
                            dl = j - 2 * g
                            wd = 128 if dl == 1 else 256
                            q_sl = (qT[b, s][:, g, 128:256] if dl == 1
                                    else qT[b, s][:, g, :])
                            nc.tensor.matmul(s2[:, lo:lo + wd],
                                             lhsT=kT_chunk(b, j), rhs=q_sl,
                                             start=True, stop=True)
                            if s >= 1 and dl <= -1:
                                info.append((lo, wd, 'bias',
                                             14 * (s - 1) + (dl + 14)))
                            else:
                                rc = ({-1: 0, 0: 256, 1: 512}[dl] if s == 0
                                      else {0: 640, 1: 896}[dl])
                                info.append((lo, wd, 'stt', rc))
                            lo += wd
                        self.expp[k] = (expp, info)
                        i = 0
                        while i < len(info):
                            lo, wd, kind, c = info[i]
                            if kind == 'bias':
                                nc.scalar.activation(
                                    expp[:, lo:lo + wd], s2[:, lo:lo + wd],
                                    Exp, bias=bias_sb[:, c:c + 1])
                                i += 1
                                continue
                            # merge stt halves contiguous in s2 AND rel cols
                            tw, i2 = wd, i + 1
                            while (i2 < len(info) and info[i2][2] == 'stt'
                                   and info[i2][0] == lo + tw
                                   and info[i2][3] == c + tw):
                                tw += info[i2][1]
                                i2 += 1
                            tmp = attn.tile([128, 512], f32, tag="stt",
                                            bufs=4, name="tmp")
                            nc.vector.scalar_tensor_tensor(
                                out=tmp[:, lo:lo + tw],
                                in0=rel_sb[:, c:c + tw],
                                scalar=slopes_sb[:, s:s + 1],
                                in1=s2[:, lo:lo + tw], op0=MULT, op1=ADD)
                            nc.scalar.activation(expp[:, lo:lo + tw],
                                                 tmp[:, lo:lo + tw], Exp)
                            i = i2

                    def emit_stage2(self, k):
                        b = self.b
                        ja, jb = self.pairs[k]
                        expp, info = self.expp.pop(k)
                        js = [j for j in (ja, jb) if j is not None]
                        for (lo, wd, _, _), j in zip(info, js):
                            e_sl = expp[:, lo:lo + wd]
                            olo = 128 if wd == 128 else 0
                            nc.tensor.matmul(
                                self.at[:, olo:olo + wd],
                                lhsT=v_b[b][:, j // 2, j % 2, :],
                                rhs=e_sl, start=(j == self.j0),
                                stop=(j == self.nch - 1))
                            nc.tensor.matmul(
                                self.den[:, olo:olo + wd], lhsT=ones_mat[:],
                                rhs=e_sl, start=(j == self.j0),
                                stop=(j == self.nch - 1))

                    def emit_tail(self):
                        b, s, g = self.b, self.s, self.g
                        rec = attn.tile([128, 256], f32, tag="rec", bufs=3,
                                        name="rec")
                        nc.vector.reciprocal_approx_fast(out=rec[:],
                                                         in_=self.den[:])
                        nc.vector.tensor_tensor(out=aT[b, s][:, g, :],
                                                in0=self.at[:], in1=rec[:],
                                                op=MULT)

                LAG = 7

                def attn_half(b, h):
                    # software pipeline: stage2 (PV/den) trails stage1 (QK/
                    # bias/exp) by LAG entries so the PE never reaches a
                    # matmul whose exp isn't long since finished
                    pend = []

                    def pop_one():
                        u, k = pend.pop(0)
                        u.emit_stage2(k)
                        if k == len(u.pairs) - 1:
                            u.emit_tail()

                    for g in range(GQ):
                        units = [AttnUnit(b, 2 * h, g),
                                 AttnUnit(b, 2 * h + 1, g)]
                        for k in range(max(len(u.pairs) for u in units)):
                            for u in units:
                                if k < len(u.pairs):
                                    u.emit_stage1(k)
                                    pend.append((u, k))
                                    while len(pend) > LAG:
                                        pop_one()
                    while pend:
                        pop_one()

                def ship_attnT(b, h):
                    # send to rank j: my slots (2h, 2h+1) attn for j's tokens
                    for si in range(2):
                        nc.sync.dma_start(
                            out=a_in[b, h].rearrange("(j s p) t -> s p j t",
                                                     s=2, p=128)[si],
                            in_=aT[b, 2 * h + si][:])
                    nc.gpsimd.collective_compute(
                        "AllToAll", mybir.AluOpType.bypass,
                        replica_groups=grp,
                        ins=[a_in[b, h][:]], outs=[a_a2a[b, h][:]])

                att_sb = {}

                def load_att_sb(b):
                    # chunk l of att_sb == head l == Wo row-chunk l
                    att_sb[b] = ph4.tile([128, KC, 256], bf16, tag="att",
                                         bufs=2, name=f"att_sb{b}")
                    for h in range(2):
                        for si in range(2):
                            base = 16 * h + 8 * si
                            nc.sync.dma_start(
                                out=att_sb[b][:, base:base + 8, :],
                                in_=a_a2a[b, h].rearrange(
                                    "(j s p) t -> s p j t", s=2, p=128)[si])

                wo_tiles = {}

                def ensure_wo(i):
                    if 0 <= i < 16 and i not in wo_tiles:
                        n8 = i % 8
                        w = ph4.tile([128, KC, 512], bf16, tag="wo", bufs=2,
                                     name="wo_sb")
                        nc.scalar.dma_start(
                            out=w[:],
                            in_=Wo_t[n8].rearrange("p (k n) -> p k n", k=KC))
                        wo_tiles[i] = w

                def outproj_unit(b, n8, th, wo_sb):
                    o_ps = psum.tile([128, 512], f32, tag="s2", bufs=3,
                                     name="o_ps")
                    for k in range(KC):
                        nc.tensor.matmul(
                            o_ps[:],
                            lhsT=att_sb[b][:, k, 128 * th:128 * (th + 1)],
                            rhs=wo_sb[:, k, :],
                            start=(k == 0), stop=(k == KC - 1))
                    o_sb = ph4.tile([128, 512], bf16, tag="ostage", bufs=2,
                                    name="o_sb")
                    nc.vector.tensor_copy(out=o_sb[:], in_=o_ps[:])
                    r0 = 256 * b + 128 * th
                    nc.sync.dma_start(
                        out=out[r0:r0 + 128, 512 * n8:512 * (n8 + 1)],
                        in_=o_sb[:])

                for b in range(B):
                    for h in range(2):
                        attn_half(b, h)
                        ship_attnT(b, h)
                        if (b, h) == (0, 1):
                            # Wo tile 0 + b0 att stream while b1 attn runs
                            ensure_wo(0)
                            load_att_sb(0)
                ensure_wo(1)
                for i in range(16):
                    b, n8 = divmod(i, 8)
                    if i == 4:
                        # b1's a2a has landed by now; emitting this late
                        # keeps the wo prefetch queue ahead of it
                        load_att_sb(1)
                    ensure_wo(i + 1)
                    for th in range(2):
                        outproj_unit(b, n8, th, wo_tiles[i])
    nc.finalize()
    return nc


def kernel(hidden_states, Wq, Wkv, Wo):
    global LAST_EXEC_NS
    bf = ml_dtypes.bfloat16
    hs = np.asarray(hidden_states, dtype=np.float32)
    Wq = np.asarray(Wq, dtype=np.float32)
    Wkv_np = np.asarray(Wkv, dtype=np.float32)
    Wo = np.asarray(Wo, dtype=np.float32)

    Wo_t = np.ascontiguousarray(
        Wo.reshape(KC, 128, 8, 512).transpose(2, 1, 0, 3)
        .reshape(8, 128, KC * 512)).astype(bf)
    Wkv_t = np.ascontiguousarray(
        Wkv_np.reshape(KC, 128, 256).transpose(1, 0, 2)
        .reshape(128, KC * 256)).astype(bf)
    rel = _build_rel()
    slopes = _alibi_slopes(NH)

    # all tokens, 512-token blocks, k-major per block: [NB, 128, KC*512]
    hs_flat = hs.reshape(B * S, H)           # [b0 2048][b1 2048]
    hsT_all = np.ascontiguousarray(
        hs_flat.reshape(NB, 512, KC, 128).transpose(0, 3, 2, 1)
        .reshape(NB, 128, KC * 512)).astype(bf)

    in_maps = []
    for c in range(NC):
        blk = np.concatenate([hs[0, 256 * c:256 * (c + 1)],
                              hs[1, 256 * c:256 * (c + 1)]], axis=0)
        hsT_c = np.ascontiguousarray(
            blk.T.reshape(KC, 128, TPC).transpose(1, 0, 2)
            .reshape(128, KC * TPC)).astype(bf)
        my_heads = [c + 8 * s for s in range(4)]
        # s-major: [4 slots][128 part][KC*128], each slot contiguous
        Wq_m = np.ascontiguousarray(np.stack([
            (Wq[:, 128 * h:128 * (h + 1)] * SCALE)
            .reshape(KC, 128, 128).transpose(1, 0, 2).reshape(128, KC * 128)
            for h in my_heads])).astype(bf)
        slopes_c = np.ascontiguousarray(
            np.broadcast_to(slopes[my_heads][None, :], (128, 4)))
        in_maps.append({
            "hsT_my": hsT_c, "hsT_all": hsT_all, "Wq_m": Wq_m,
            "Wkv": Wkv_t, "Wo_t": Wo_t, "rel": rel, "slopes": slopes_c,
            "biast": _build_bias(slopes[my_heads]),
        })

    if "nc" not in _CACHE:
        _CACHE["nc"] = _build_nc()
    nc = _CACHE["nc"]
    trace = bool(int(os.environ.get("BASS_KERNEL_TRACE", "0")))
    res = run_bass_kernel_spmd(nc, in_maps, core_ids=list(range(NC)),
                               trace=trace)
    LAST_EXEC_NS = res.exec_time_ns
    out_full = np.empty((B, S, H), np.float32)
    for c in range(NC):
        oc = np.asarray(res.results[c]["out"], dtype=np.float32)
        out_full[0, 256 * c:256 * (c + 1)] = oc[0:256]
        out_full[1, 256 * c:256 * (c + 1)] = oc[256:512]
    return out_full


# revision 15
# speedup vs baseline: 1.0135x; 1.0135x over previous
"""MQA attention with ALiBi + causal mask on 8 TRN2 NeuronCores.

Problem: hidden_states [2,2048,4096] @ Wq -> 32 query heads of 128; single
KV head via Wkv; scores + ALiBi bias + causal mask; softmax; @ Wo.

Distribution: q projection and attention are HEAD-sharded (core c owns
heads {c, c+8, c+16, c+24}, one per octile "slot"), so there is NO q
AllToAll: each core computes q for its 4 heads over ALL 4096 tokens
(same FLOPs as token-sharded q). KV projection and the output projection
are TOKEN-sharded (core c owns 256 tokens of each batch). Bridges: tiny
kT/v AllGathers right after the kv projection, and one attnT AllToAll
per (batch, head-half) fired the moment those 16 heads finish, so every
collective hides under compute and the output projection never waits.

Attention: scoresT[kv, q] layout; per-slot ALiBi reach cutoff at
threshold 14 (dropped tail mass ~8e-7): SLOT_D = [56, 224, 896, inf].
Softmax shift-invariance: the per-q-column ALiBi term exp(-slope*f)
cancels between numerator and denominator, so off-diagonal chunks of
slots 1-3 need only a PER-PARTITION bias slope*(128*delta+p), folded
into the exp activation's bias operand -- no DVE bias op at all. Slot 0
(slopes up to 0.84, column shift up to e^214 would overflow f32) keeps
the exact rel path; diagonal chunks use a masked, -f-free rel' table so
they stay consistent with the bias-path chunks of the same head.
exp without max-subtraction (scores O(10), shifted args bounded by
~e^80 < f32 max); denominator via ALL-ONES [128,128] matmuls whose
output rows are all equal to den -- the weight load uses the normal
full-array FWL path (a [128,1] ones column forced a col_grp=q0 partial
load that serialized against in-flight matmuls, ~100ns/matmul), and the
partition-broadcast of 1/den comes free: reciprocal + multiply on DVE,
no broadcast matmul, no scalar-engine staging copy. The PV/den matmuls
trail the QK matmuls by LAG=7 pipeline entries (3-deep PSUM
accumulators): the PE queue is FIFO, so a PV waiting on its exp would
stall every matmul (and its weight load) queued behind it.

All matmuls bf16 (rel-err ~6e-3 vs budget 2e-2); output staged bf16 and
cast to f32 on the host. Weights/activations are host-pre-tiled so every
DMA is contiguous; the q-projection's block-0 DMAs are issued first and
its matmuls emitted before the kv projection, so the PE starts ~1us in
and the kv DMAs hide behind block-0 compute. PSUM: s2/qproj/outproj
share a 3-buffer [128,512] tag; at/den per-unit accumulators are
[128,256] each with 3 buffers. Wo tile 0 prefetches during attention;
outproj runs an explicit i+1 prefetch chain.
"""
import math
import os

import numpy as np
import ml_dtypes

import concourse.bass as bass
from concourse import bacc
import concourse.mybir as mybir
from concourse.tile import TileContext
from concourse.bass_utils import run_bass_kernel_spmd

B, S, H, NH, HD = 2, 2048, 4096, 32, 128
NC = 8              # cores
TPC = 512           # tokens per core (256 per batch) for kv/out projections
KC = H // 128       # 32 contraction chunks
GQ = 8              # 256-token q blocks per batch
NB = 8              # 512-token blocks across both batches (q projection)
SCALE = HD ** -0.5
# per-slot ALiBi reach: 14/min_slope(octile); tail mass <= exp(-14) ~ 8e-7
SLOT_D = [56.0, 224.0, 896.0, float("inf")]
bf16 = mybir.dt.bfloat16
f32 = mybir.dt.float32
Exp = mybir.ActivationFunctionType.Exp
Copy = mybir.ActivationFunctionType.Copy
MULT = mybir.AluOpType.mult
ADD = mybir.AluOpType.add

_CACHE = {}
LAST_EXEC_NS = None


def _alibi_slopes(n_heads):
    closest_pow2 = 2 ** math.floor(math.log2(n_heads))
    base = 2.0 ** (-(2.0 ** -(math.log2(closest_pow2) - 3)))
    slopes = [base ** i for i in range(1, closest_pow2 + 1)]
    if closest_pow2 != n_heads:
        extra_base = 2.0 ** (-(2.0 ** -(math.log2(2 * closest_pow2) - 3)))
        n_extra = min(closest_pow2, n_heads - closest_pow2)
        slopes += [extra_base ** i for i in range(1, 2 * n_extra + 1, 2)]
    return np.asarray(slopes, dtype=np.float32)


def _j0(g, slot):
    d = SLOT_D[slot]
    if math.isinf(d):
        return 0
    return max(0, math.ceil((256 * g - 127 - d) / 128))


def _build_rel():
    """5 slices of 256 cols: idx 0..2 = exact rel (delta=-1,0,1) for the
    slot-0 path; idx 3..4 = -f-free rel' (delta=0,1) for slot>=1 diagonal
    chunks (value 128*delta+p, causal mask -30000)."""
    rel = np.empty((128, 5 * 256), np.float32)
    p = np.arange(128)[:, None]
    f = np.arange(256)[None, :]
    for i, delta in enumerate((-1, 0, 1)):
        r = (128 * delta + p - f).astype(np.float32)
        r[128 * delta + p - f > 0] = -30000.0
        rel[:, 256 * i:256 * (i + 1)] = r
    for i, delta in enumerate((0, 1)):
        r = np.broadcast_to((128 * delta + p).astype(np.float32),
                            (128, 256)).copy()
        r[128 * delta + p - f > 0] = -30000.0
        rel[:, 256 * (3 + i):256 * (4 + i)] = r
    return rel.astype(ml_dtypes.bfloat16)


def _build_bias(slopes4):
    """[128, 42] f32: col 14*(s-1)+(delta+14) = slope_s*(128*delta+p) for
    slots 1..3, delta in [-14, -1] (off-diagonal bias-path chunks)."""
    b = np.zeros((128, 42), np.float32)
    p = np.arange(128)
    for s in (1, 2, 3):
        for d in range(-14, 0):
            b[:, 14 * (s - 1) + (d + 14)] = slopes4[s] * (128 * d + p)
    return b


def _build_nc():
    nc = bacc.Bacc(num_devices=NC)
    # host-pre-tiled layouts: every weight/act DMA is contiguous in DRAM
    hsT_my = nc.declare_dram_parameter("hsT_my", [128, KC * TPC], bf16,
                                       isOutput=False)
    hsT_all = nc.declare_dram_parameter("hsT_all", [NB, 128, KC * 512], bf16,
                                        isOutput=False)
    Wq_m = nc.declare_dram_parameter("Wq_m", [4, 128, KC * 128], bf16,
                                     isOutput=False)
    Wkv = nc.declare_dram_parameter("Wkv", [128, KC * 256], bf16,
                                    isOutput=False)
    Wo_t = nc.declare_dram_parameter("Wo_t", [8, 128, KC * 512], bf16,
                                     isOutput=False)
    rel = nc.declare_dram_parameter("rel", [128, 5 * 256], bf16,
                                    isOutput=False)
    slopes = nc.declare_dram_parameter("slopes", [128, 4], f32,
                                       isOutput=False)
    biast = nc.declare_dram_parameter("biast", [128, 42], f32,
                                      isOutput=False)
    out = nc.declare_dram_parameter("out", [TPC, H], bf16, isOutput=True)

    grp = [list(range(NC))]
    with TileContext(nc) as tc:
        with (
            tc.tile_pool(name="dram", bufs=1, space="DRAM") as dram,
            tc.tile_pool(name="const", bufs=1) as const,
            tc.tile_pool(name="psum", bufs=1, space="PSUM") as psum,
            tc.tile_pool(name="lng", bufs=1) as lng,
        ):
            kT_in = dram.tile([128, TPC], bf16)
            kT_ag = dram.tile([128 * NC, TPC], bf16, addr_space="Shared")
            v_in = dram.tile([TPC, 128], bf16)
            v_ag = dram.tile([TPC * NC, 128], bf16, addr_space="Shared")
            # attnT bounce buffers per (batch, head-half)
            a_in = {(b, h): dram.tile([2048, 256], bf16, name=f"a_in{b}{h}")
                    for b in range(2) for h in range(2)}
            a_a2a = {(b, h): dram.tile([2048, 256], bf16, name=f"a_a2a{b}{h}")
                     for b in range(2) for h in range(2)}

            ones_mat = const.tile([128, 128], bf16)
            nc.vector.memset(ones_mat[:], 1.0)

            # ---------------- Phase 1+2: projections ----------------------
            with tc.tile_pool(name="ph1", bufs=1) as ph1:
                # Startup DMAs ride both HW DGE queues: activations on the
                # Sync queue, weights on the Scalar queue (idle until the
                # attention exps), ordered by PE consumption. Wq is s-major
                # so the first matmul group needs only its own 1MB slice.
                Wq_sb = ph1.tile([128, 4, KC, 128], bf16)
                hs0 = ph1.tile([128, KC, 512], bf16, tag="hsblk", bufs=2,
                               name="hs_blk")
                hs_r0 = hsT_all[0].rearrange("p (k t) -> p k t", k=KC)
                for lo, hi in ((0, 2), (2, 4), (4, 8), (8, 16), (16, 32)):
                    nc.sync.dma_start(out=hs0[:, lo:hi, :],
                                      in_=hs_r0[:, lo:hi, :])
                for s in range(4):
                    nc.scalar.dma_start(
                        out=Wq_sb[:, s],
                        in_=Wq_m[s].rearrange("p (k m) -> p k m", k=KC))
                # kv-projection DMAs queue behind and hide under block 0
                hsT_sb = ph1.tile([128, KC, TPC], bf16)
                Wkv_sb = ph1.tile([128, KC, 256], bf16)
                hsT_r = hsT_my.rearrange("p (k t) -> p k t", k=KC)
                Wkv_r = Wkv.rearrange("p (k c) -> p k c", k=KC)
                nc.scalar.dma_start(out=Wkv_sb[:], in_=Wkv_r[:])
                for lo, hi in ((0, 8), (8, 20), (20, 32)):
                    sl = slice(lo, hi)
                    nc.sync.dma_start(out=hsT_sb[:, sl, :], in_=hsT_r[:, sl, :])
                rel_sb = const.tile([128, 5 * 256], bf16)
                slopes_sb = const.tile([128, 4], f32)
                bias_sb = const.tile([128, 42], f32)

                # qT[b, s]: [128 d, 8 g-blocks, 256 tok]
                qT = {(b, s): lng.tile([128, GQ, 256], bf16,
                                       name=f"qT_{b}_{s}")
                      for b in range(2) for s in range(4)}
                kT_b, v_b = {}, {}

                def load_kv_sbuf():
                    nc.scalar.dma_start(out=rel_sb[:], in_=rel[:])
                    nc.scalar.dma_start(out=slopes_sb[:], in_=slopes[:])
                    nc.scalar.dma_start(out=bias_sb[:], in_=biast[:])
                    for bb in range(B):
                        t = lng.tile([128, 8, 256], bf16, name=f"kT_{bb}")
                        nc.scalar.dma_start(
                            out=t[:],
                            in_=kT_ag.rearrange("(r p) (b t) -> b p r t",
                                                p=128, b=2)[bb])
                        kT_b[bb] = t
                        t = lng.tile([128, 8, 2, 128], bf16, name=f"v_{bb}")
                        for u in range(2):
                            nc.scalar.dma_start(
                                out=t[:, :, u, :],
                                in_=v_ag.rearrange(
                                    "(r b u p) d -> b p r u d",
                                    b=2, u=2, p=128)[bb][:, :, u, :])
                        v_b[bb] = t

                def qproj_block(tb, hs_blk):
                    b = tb // 4
                    g2 = tb % 4
                    for s in range(4):
                        q_ps = psum.tile([128, 512], f32, tag="s2", bufs=3,
                                         name="q_ps")
                        for k in range(KC):
                            nc.tensor.matmul(
                                q_ps[:],
                                lhsT=Wq_sb[:, s, k, :],
                                rhs=hs_blk[:, k, :],
                                start=(k == 0), stop=(k == KC - 1))
                        nc.vector.tensor_copy(
                            out=qT[b, s][:, 2 * g2:2 * g2 + 2, :],
                            in_=q_ps[:])

                # block 0 first: PE busy while the kv DMAs stream in
                qproj_block(0, hs0)

                # kv projection (my 512 tokens) + AllGathers
                kT_ps = psum.tile([128, TPC], f32, tag="s2", bufs=3)
                for k in range(KC):
                    nc.tensor.matmul(kT_ps[:], lhsT=Wkv_sb[:, k, 0:128],
                                     rhs=hsT_sb[:, k, :],
                                     start=(k == 0), stop=(k == KC - 1))
                kT_sb = ph1.tile([128, TPC], bf16)
                nc.vector.tensor_copy(out=kT_sb[:], in_=kT_ps[:])
                nc.sync.dma_start(out=kT_in[:], in_=kT_sb[:])
                nc.gpsimd.collective_compute(
                    "AllGather", mybir.AluOpType.bypass, replica_groups=grp,
                    ins=[kT_in[:]], outs=[kT_ag[:]])

                for t4 in range(4):
                    v_ps = psum.tile([128, 128], f32, tag="s2", bufs=3,
                                     name="v_ps")
                    for k in range(KC):
                        nc.tensor.matmul(
                            v_ps[:],
                            lhsT=hsT_sb[:, k, 128 * t4:128 * (t4 + 1)],
                            rhs=Wkv_sb[:, k, 128:256],
                            start=(k == 0), stop=(k == KC - 1))
                    v_sb = ph1.tile([128, 128], bf16, tag="v_sb", bufs=3,
                                    name="v_sb")
                    nc.vector.tensor_copy(out=v_sb[:], in_=v_ps[:])
                    nc.sync.dma_start(out=v_in[128 * t4:128 * (t4 + 1), :],
                                      in_=v_sb[:])
                nc.gpsimd.collective_compute(
                    "AllGather", mybir.AluOpType.bypass, replica_groups=grp,
                    ins=[v_in[:]], outs=[v_ag[:]])

                # remaining q-projection blocks
                for tb in range(1, NB):
                    if tb == 6:
                        load_kv_sbuf()
                    hs_blk = ph1.tile([128, KC, 512], bf16, tag="hsblk",
                                      bufs=2, name="hs_blk")
                    nc.sync.dma_start(
                        out=hs_blk[:],
                        in_=hsT_all[tb].rearrange("p (k t) -> p k t", k=KC))
                    qproj_block(tb, hs_blk)

            # ---------------- Phases 3+4: attention & output projection ----
            with (tc.tile_pool(name="attn", bufs=1) as attn,
                  tc.tile_pool(name="ph4", bufs=1) as ph4):
                aT = {}
                for b in range(B):
                    for s in range(4):
                        aT[b, s] = attn.tile([128, GQ, 256], bf16, tag="aT",
                                             bufs=8, name=f"aT_{b}_{s}")

                def kT_chunk(b, j):
                    return kT_b[b][:, j // 2, 128 * (j % 2):128 * (j % 2 + 1)]

                class AttnUnit:
                    """One (b, s, g) softmax unit, emitted in two stages so
                    the PV/den matmuls trail the QK matmuls by several
                    entries: the PE queue is FIFO, so a PV waiting on its
                    exp would otherwise stall every matmul (and its weight
                    load) queued behind it."""

                    def __init__(self, b, s, g):
                        self.b, self.s, self.g = b, s, g
                        self.nch = 2 * (g + 1)
                        self.j0 = _j0(g, s)
                        js = list(range(self.j0, self.nch))
                        self.pairs = [
                            (js[i], js[i + 1] if i + 1 < len(js) else None)
                            for i in range(0, len(js), 2)]
                        self.expp = {}
                        self.at = psum.tile([128, 256], f32, tag="at",
                                            bufs=3, name="at")[:]
                        self.den = psum.tile([128, 256], f32, tag="den",
                                             bufs=2, name="den")[:]

                    def emit_stage1(self, k):
                        b, s, g = self.b, self.s, self.g
                        ja, jb = self.pairs[k]
                        w = 512 if jb is not None else 256
                        s2 = psum.tile([128, 512], f32, tag="s2", bufs=3,
                                       name="s2")
                        nc.tensor.matmul(s2[:, 0:256], lhsT=kT_chunk(b, ja),
                                         rhs=qT[b, s][:, g, :],
                                         start=True, stop=True)
                        if jb is not None:
                            nc.tensor.matmul(s2[:, 256:512],
                                             lhsT=kT_chunk(b, jb),
                                             rhs=qT[b, s][:, g, :],
                                             start=True, stop=True)
                        expp = attn.tile([128, 512], bf16, tag="exp", bufs=11,
                                         name="expp")
                        self.expp[k] = (expp, w)
                        halves = [(0, ja)] + ([(1, jb)] if jb is not None
                                              else [])
                        # stt path: slot 0 (exact rel, no column shift) and
                        # diagonal chunks (masked rel'); bias path otherwise
                        stt_h = [(i, j) for i, j in halves
                                 if s == 0 or j - 2 * g >= 0]
                        bias_h = [(i, j) for i, j in halves
                                  if not (s == 0 or j - 2 * g >= 0)]
                        for i, j in bias_h:
                            col = 14 * (s - 1) + (j - 2 * g + 14)
                            nc.scalar.activation(
                                expp[:, 256 * i:256 * (i + 1)],
                                s2[:, 256 * i:256 * (i + 1)], Exp,
                                bias=bias_sb[:, col:col + 1])
                        if stt_h:
                            tmp = attn.tile([128, 512], f32, tag="stt",
                                            bufs=4, name="tmp")
                            i0, ja0 = stt_h[0]
                            idx = ((ja0 - 2 * g + 1) if s == 0
                                   else (3 + ja0 - 2 * g))
                            ws = 256 * len(stt_h)
                            lo = 256 * i0
                            nc.vector.scalar_tensor_tensor(
                                out=tmp[:, lo:lo + ws],
                                in0=rel_sb[:, 256 * idx:256 * idx + ws],
                                scalar=slopes_sb[:, s:s + 1],
                                in1=s2[:, lo:lo + ws], op0=MULT, op1=ADD)
                            nc.scalar.activation(expp[:, lo:lo + ws],
                                                 tmp[:, lo:lo + ws], Exp)

                    def emit_stage2(self, k):
                        b = self.b
                        ja, jb = self.pairs[k]
                        expp, w = self.expp.pop(k)
                        for ji, j in ((0, ja), (1, jb)):
                            if j is None:
                                continue
                            e_sl = expp[:, 256 * ji:256 * (ji + 1)]
                            nc.tensor.matmul(
                                self.at, lhsT=v_b[b][:, j // 2, j % 2, :],
                                rhs=e_sl, start=(j == self.j0),
                                stop=(j == self.nch - 1))
                            nc.tensor.matmul(
                                self.den, lhsT=ones_mat[:], rhs=e_sl,
                                start=(j == self.j0),
                                stop=(j == self.nch - 1))

                    def emit_tail(self):
                        b, s, g = self.b, self.s, self.g
                        rec = attn.tile([128, 256], f32, tag="rec", bufs=3,
                                        name="rec")
                        nc.vector.reciprocal_approx_fast(out=rec[:],
                                                         in_=self.den)
                        nc.vector.tensor_tensor(out=aT[b, s][:, g, :],
                                                in0=self.at, in1=rec[:],
                                                op=MULT)

                LAG = 7

                def attn_half(b, h):
                    # software pipeline: stage2 (PV/den) trails stage1 (QK/
                    # bias/exp) by LAG entries so the PE never reaches a
                    # matmul whose exp isn't long since finished
                    pend = []

                    def pop_one():
                        u, k = pend.pop(0)
                        u.emit_stage2(k)
                        if k == len(u.pairs) - 1:
                            u.emit_tail()

                    for g in range(GQ):
                        units = [AttnUnit(b, 2 * h, g),
                                 AttnUnit(b, 2 * h + 1, g)]
                        for k in range(max(len(u.pairs) for u in units)):
                            for u in units:
                                if k < len(u.pairs):
                                    u.emit_stage1(k)
                                    pend.append((u, k))
                                    while len(pend) > LAG:
                                        pop_one()
                    while pend:
                        pop_one()

                def ship_attnT(b, h):
                    # send to rank j: my slots (2h, 2h+1) attn for j's tokens
                    for si in range(2):
                        nc.sync.dma_start(
                            out=a_in[b, h].rearrange("(j s p) t -> s p j t",
                                                     s=2, p=128)[si],
                            in_=aT[b, 2 * h + si][:])
                    nc.gpsimd.collective_compute(
                        "AllToAll", mybir.AluOpType.bypass,
                        replica_groups=grp,
                        ins=[a_in[b, h][:]], outs=[a_a2a[b, h][:]])

                att_sb = {}

                def load_att_sb(b):
                    # chunk l of att_sb == head l == Wo row-chunk l
                    att_sb[b] = ph4.tile([128, KC, 256], bf16, tag="att",
                                         bufs=2, name=f"att_sb{b}")
                    for h in range(2):
                        for si in range(2):
                            base = 16 * h + 8 * si
                            nc.sync.dma_start(
                                out=att_sb[b][:, base:base + 8, :],
                                in_=a_a2a[b, h].rearrange(
                                    "(j s p) t -> s p j t", s=2, p=128)[si])

                wo_tiles = {}

                def ensure_wo(i):
                    if 0 <= i < 16 and i not in wo_tiles:
                        n8 = i % 8
                        w = ph4.tile([128, KC, 512], bf16, tag="wo", bufs=2,
                                     name="wo_sb")
                        nc.scalar.dma_start(
                            out=w[:],
                            in_=Wo_t[n8].rearrange("p (k n) -> p k n", k=KC))
                        wo_tiles[i] = w

                def outproj_unit(b, n8, th, wo_sb):
                    o_ps = psum.tile([128, 512], f32, tag="s2", bufs=3,
                                     name="o_ps")
                    for k in range(KC):
                        nc.tensor.matmul(
                            o_ps[:],
                            lhsT=att_sb[b][:, k, 128 * th:128 * (th + 1)],
                            rhs=wo_sb[:, k, :],
                            start=(k == 0), stop=(k == KC - 1))
                    o_sb = ph4.tile([128, 512], bf16, tag="ostage", bufs=2,
                                    name="o_sb")
                    nc.vector.tensor_copy(out=o_sb[:], in_=o_ps[:])
                    r0 = 256 * b + 128 * th
                    nc.sync.dma_start(
                        out=out[r0:r0 + 128, 512 * n8:512 * (n8 + 1)],
                        in_=o_sb[:])

                for b in range(B):
                    for h in range(2):
                        attn_half(b, h)
                        ship_attnT(b, h)
                        if (b, h) == (0, 1):
                            # Wo tile 0 + b0 att stream while b1 attn runs
                            ensure_wo(0)
                            load_att_sb(0)
                ensure_wo(1)
                for i in range(16):
                    b, n8 = divmod(i, 8)
                    if i == 4:
                        # b1's a2a has landed by now; emitting this late
                        # keeps the wo prefetch queue ahead of it
                        load_att_sb(1)
                    ensure_wo(i + 1)
                    for th in range(2):
                        outproj_unit(b, n8, th, wo_tiles[i])
    nc.finalize()
    return nc


def kernel(hidden_states, Wq, Wkv, Wo):
    global LAST_EXEC_NS
    bf = ml_dtypes.bfloat16
    hs = np.asarray(hidden_states, dtype=np.float32)
    Wq = np.asarray(Wq, dtype=np.float32)
    Wkv_np = np.asarray(Wkv, dtype=np.float32)
    Wo = np.asarray(Wo, dtype=np.float32)

    Wo_t = np.ascontiguousarray(
        Wo.reshape(KC, 128, 8, 512).transpose(2, 1, 0, 3)
        .reshape(8, 128, KC * 512)).astype(bf)
    Wkv_t = np.ascontiguousarray(
        Wkv_np.reshape(KC, 128, 256).transpose(1, 0, 2)
        .reshape(128, KC * 256)).astype(bf)
    rel = _build_rel()
    slopes = _alibi_slopes(NH)

    # all tokens, 512-token blocks, k-major per block: [NB, 128, KC*512]
    hs_flat = hs.reshape(B * S, H)           # [b0 2048][b1 2048]
    hsT_all = np.ascontiguousarray(
        hs_flat.reshape(NB, 512, KC, 128).transpose(0, 3, 2, 1)
        .reshape(NB, 128, KC * 512)).astype(bf)

    in_maps = []
    for c in range(NC):
        blk = np.concatenate([hs[0, 256 * c:256 * (c + 1)],
                              hs[1, 256 * c:256 * (c + 1)]], axis=0)
        hsT_c = np.ascontiguousarray(
            blk.T.reshape(KC, 128, TPC).transpose(1, 0, 2)
            .reshape(128, KC * TPC)).astype(bf)
        my_heads = [c + 8 * s for s in range(4)]
        # s-major: [4 slots][128 part][KC*128], each slot contiguous
        Wq_m = np.ascontiguousarray(np.stack([
            (Wq[:, 128 * h:128 * (h + 1)] * SCALE)
            .reshape(KC, 128, 128).transpose(1, 0, 2).reshape(128, KC * 128)
            for h in my_heads])).astype(bf)
        slopes_c = np.ascontiguousarray(
            np.broadcast_to(slopes[my_heads][None, :], (128, 4)))
        in_maps.append({
            "hsT_my": hsT_c, "hsT_all": hsT_all, "Wq_m": Wq_m,
            "Wkv": Wkv_t, "Wo_t": Wo_t, "rel": rel, "slopes": slopes_c,
            "biast": _build_bias(slopes[my_heads]),
        })

    if "nc" not in _CACHE:
        _CACHE["nc"] = _build_nc()
    nc = _CACHE["nc"]
    trace = bool(int(os.environ.get("BASS_KERNEL_TRACE", "0")))
    res = run_bass_kernel_spmd(nc, in_maps, core_ids=list(range(NC)),
                               trace=trace)
    LAST_EXEC_NS = res.exec_time_ns
    out_full = np.empty((B, S, H), np.float32)
    for c in range(NC):
        oc = np.asarray(res.results[c]["out"], dtype=np.float32)
        out_full[0, 256 * c:256 * (c + 1)] = oc[0:256]
        out_full[1, 256 * c:256 * (c + 1)] = oc[256:512]
    return out_full
